# revision 40
# baseline (speedup 1.0000x reference)
"""Multi-head self-attention on 8 trn2 NeuronCores.

Problem: x[2,2048,1024], 16 heads, depth 64; out = MHA(x) with QKV/O
projections (reference.py / nn_MultiHeadSelfAttention_3341484556968).

Sharding: tensor-parallel over heads. Core c owns heads {2c, 2c+1} (128
features). Per core:
  - QKV projections for its heads in T-layout ([feat, rows]); x and the
    QKV/O weights stream in as bf16, outputs kept f32r.
  - Scores computed transposed ([k, q]) so softmax sits on the partition
    axis; the two heads are row-packed on the PE via tile_position (K=64
    each). exp on ScalarE with the 1/sqrt(depth) scale folded in.
  - PV runs with the exp weights as the STATIONARY operand ([k,q]-chunk)
    and [1|V] (65 cols) moving, producing attn in [q, f] orientation at
    half the PE cost of the [f, q] orientation (the 65-wide operand no
    longer wastes half the PE), with the softmax denominator accumulating
    in the ones column.
  - Normalization is then a per-partition scalar multiply (q on
    partitions): one DVE reciprocal + 8 small scalar-muls per q-block,
    writing bf16 staging tiles directly (no ones-broadcast matmuls, no
    psum evictions).
  - Reshard (head-split -> row-split) uses a few grouped AllToAll
    collectives (the cost model charges ~15us fixed per collective, all
    serialized on the collective cores, so fewer+bigger wins; the last
    group is a single chunk to minimize the tail).
  - The receiver loads each chunk with a transposing DMA (XBAR), which
    flips the [q, f] payload to the [f, q] orientation the output
    projection wants. Output projection (bf16, weights resident) runs
    granule-wise as in the baseline, with dummy "warmer" matmuls keeping
    the PE p-state ramped through collective waits.
Host re-interleaves the per-core [1024, 512] outputs.

V is projected directly in [rows, feat] orientation and the V bias folds
into the output bias on the host (softmax weights sum to 1). Score
matmuls run float32r; everything else bf16.
"""

import os

import ml_dtypes
import numpy as np

import concourse.bacc as bacc
import concourse.mybir as mybir
import concourse.tile as tile

F32 = mybir.dt.float32
F32R = mybir.dt.float32r
BF16 = mybir.dt.bfloat16
AF = mybir.ActivationFunctionType

P = 128          # partitions / PE contraction width

# collective grouping over the 8 (b,qc) chunks, in chunk order
CGROUPS = (4, 2, 2)


def build_nc(B=2, S=2048, D=1024, H=16, ncores=8):
    DEP = D // H                 # head depth (64)
    HPC = H // ncores            # heads per core (2)
    FPC = HPC * DEP              # features per core (128)
    R = B * S                    # flattened rows (4096)
    RC = R // ncores             # output rows per core (512)
    KD = D // P                  # contraction chunks for projections (8)
    RWC = min(512, S)            # row chunk for projections (per batch)
    QCH = min(512, S)            # query columns per block
    NQC = S // QCH               # q blocks per batch
    NKC = S // P                 # key chunks per batch
    NDO = D // P                 # output-feature chunks (8)
    NCH = B * NQC                # total (b,qc) chunks (8)
    GR = QCH // ncores           # rows per (chunk, dst) granule (64)
    NQS = QCH // P               # q sub-chunks per reshard chunk (4)
    QB = 256                     # attention query block (psum-bank sized)
    NQB = S // QB                # attention blocks per batch (8)
    assert FPC == P
    scale = 1.0 / np.sqrt(DEP)
    # DVE fast-exp: exp(s*scale) ~= bitcast_bf16(i16(s*FE_A + FE_B)); the
    # i16 is linear in (exponent|mantissa), i.e. a mantissa-linear exp
    # approximation at bf16 resolution. FE_B centers the log-domain error
    # (mean of ln((1+f)/2^f) = 0.0397); softmax normalization cancels the
    # common mode, only the +-4% ripple vs the ACT-exp chunks remains
    # (measured 5.1e-3 rel err vs the 2e-2 gate).
    FE_A = 128.0 * 1.4426950408889634 * scale
    FE_B = 127.0 * 128.0 - 128.0 * 0.0573
    DVE_KCS = (5, 7, 9, 11, 13, 15)   # alternating with ACT chunks

    nc = bacc.Bacc("TRN2", target_bir_lowering=False, debug=False,
                   num_devices=ncores)

    xT = nc.dram_tensor("xT", [D, R], BF16, kind="ExternalInput")
    wqkvT = nc.dram_tensor("wqkvT", [D, 3 * FPC], BF16,
                            kind="ExternalInput")
    bqkv = nc.dram_tensor("bqkv", [FPC, 3], F32, kind="ExternalInput")
    woT = nc.dram_tensor("woT", [D, D], BF16, kind="ExternalInput")
    bo = nc.dram_tensor("bo", [P, NDO], F32, kind="ExternalInput")
    outT = nc.dram_tensor("outT", [D, RC], F32, kind="ExternalOutput")

    groups = []
    c0 = 0
    for gsz in CGROUPS:
        groups.append(list(range(c0, c0 + gsz)))
        c0 += gsz
    assert c0 == NCH
    grp_of = {}
    for gi, g in enumerate(groups):
        for k, ch in enumerate(g):
            grp_of[ch] = (gi, k)

    with tile.TileContext(nc) as tc:
        with (
            tc.tile_pool(name="persist", bufs=1) as persist,
            tc.tile_pool(name="stream", bufs=2) as stream,
            tc.tile_pool(name="work", bufs=2) as work,
            tc.tile_pool(name="dram", bufs=1, space="DRAM") as dram,
        ):
            # ---- constants / weights resident in SBUF ----
            wqkv_sb = persist.tile([P, KD, 3 * FPC], BF16)
            wqkv_src = wqkvT.ap().rearrange("(ko p) m -> p ko m", p=P)
            bqkv_sb = persist.tile([FPC, 3], F32)
            bo_sb = persist.tile([P, NDO], F32)
            wo_sb = persist.tile([P, NDO, D], BF16)

            QT_sb = persist.tile([P, R], F32R)
            KT_sb = persist.tile([P, R], F32R)
            # V per (batch-kc, head): [den-ones | 64 vals] = 65 cols
            V_sb = persist.tile([P, B * NKC * HPC, DEP + 1], BF16)

            # a2a staging, one dram tensor per group:
            # in : [phi 2, jj 4, chunk, ll 16, s 4, f 128]  (dst = (phi,jj))
            # out: [src 8, chunk, 1024 rows-ish]  = same byte count
            # unique tags: the dep tracker keys slots by tag, and a shared
            # default tag would serialize every reader behind every writer
            a2a_in = [dram.tile([2, 4, len(g), 16, NQS, FPC], BF16,
                                name=f"a2a_in_{gi}", tag=f"a2ai{gi}")
                      for gi, g in enumerate(groups)]
            a2a_out = [dram.tile([ncores, len(g), GR, FPC], BF16,
                                 name=f"a2a_out_{gi}", tag=f"a2ao{gi}")
                       for gi, g in enumerate(groups)]

            NRWB = S // RWC           # projection row-chunks per batch (4)
            NTB = S // P              # V chunks per batch (16)
            KCG = NKC // NRWB         # key chunks per proj chunk (4)
            TBG = NTB // NRWB
            psd = tc.tile_pool(name="ps_bcd", bufs=1, space="PSUM")
            ps = psd.__enter__()
            nc.vector.memset(V_sb[:, :, 0:1], 1.0)
            xs_tiles = {}

            def emit_proj(b, rwb, js=(0, 1)):
                r0 = b * S + rwb * RWC
                if (b, rwb) not in xs_tiles:
                    xs = stream.tile([P, KD, RWC], BF16, tag="xs", bufs=6,
                                     name=f"xs_{b}_{rwb}")
                    src = xT.ap()[:, r0:r0 + RWC].rearrange(
                        "(ko p) n -> p ko n", p=P)
                    if b == 0 and rwb == 0:
                        # interleave the weight chunk + x chunk loads so the
                        # ko-th matmul can start as soon as its pair lands
                        for ko in range(0, KD, 2):
                            nc.sync.dma_start(wqkv_sb[:, ko:ko + 2, :],
                                              wqkv_src[:, ko:ko + 2, :])
                            if ko == 0:
                                nc.sync.dma_start(bqkv_sb, bqkv.ap())
                            nc.sync.dma_start(xs[:, ko:ko + 2, :],
                                              src[:, ko:ko + 2, :])
                        nc.sync.dma_start(bo_sb, bo.ap())
                    else:
                        nc.sync.dma_start(xs, src)
                    xs_tiles[(b, rwb)] = xs
                xs = xs_tiles[(b, rwb)]
                dsts = (QT_sb, KT_sb)
                for j in js:
                    dst = dsts[j]
                    pq = ps.tile([P, RWC], F32, tag="aux", bufs=2,
                                 name=f"psqkv_{b}_{rwb}_{j}")
                    for ko in range(KD):
                        nc.tensor.matmul(
                            pq,
                            wqkv_sb[:, ko, j * FPC:(j + 1) * FPC],
                            xs[:, ko, :],
                            start=(ko == 0), stop=(ko == KD - 1))
                    nc.vector.tensor_scalar_add(
                        dst[:, r0:r0 + RWC], pq, bqkv_sb[:, j:j + 1])

            def emit_trans(b, tb):
                # V projected in [rows, feat] orientation: the 128-row x
                # chunk is stationary, V weights move. Rows land on
                # partitions = keys, exactly what transposed PV wants.
                rwb = tb // TBG
                xs = xs_tiles[(b, rwb)]
                c0 = (tb % TBG) * P
                pv = ps.tile([P, RWC], F32, tag="aux", bufs=2,
                             name=f"vdp_{b}_{tb}")
                for ko in range(KD):
                    nc.tensor.matmul(
                        pv[:, 0:FPC],
                        xs[:, ko, c0:c0 + P],
                        wqkv_sb[:, ko, 2 * FPC:3 * FPC],
                        start=(ko == 0), stop=(ko == KD - 1))
                with nc.allow_low_precision(reason="bf16 V path"):
                    for h in range(HPC):
                        nc.vector.tensor_copy(
                            V_sb[:, (b * NKC + tb) * HPC + h, 1:DEP + 1],
                            pv[:, h * DEP:(h + 1) * DEP])

            attn_tiles = {}
            exp_tiles = {}
            an_tiles = {}

            def emit_attn(b, blk, kc_lo, kc_hi, pv=True, pv_from=None):
                g0 = b * S + blk * QB
                if kc_lo == 0:
                    # one single-bank psum tile per block; q sub-chunk c in
                    # slot c; slot layout (130 f32):
                    #   [denA | Avals 64 | denB | Bvals 64]
                    attn_tiles[(b, blk)] = ps.tile(
                        [P, 2, 130], F32, tag="attn", bufs=2,
                        name=f"attn_{b}_{blk}")
                for kc in range(kc_lo, kc_hi):
                    k0 = b * S + kc * P
                    sc = ps.tile([P, 2 * QB], F32, tag="sc", bufs=4,
                                 name=f"sc_{b}_{blk}_{kc}")
                    nc.tensor.matmul(
                        sc[:, 0:QB],
                        KT_sb[0:DEP, k0:k0 + P],
                        QT_sb[0:DEP, g0:g0 + QB],
                        start=True, stop=True, tile_position=(0, 0))
                    nc.tensor.matmul(
                        sc[:, QB:2 * QB],
                        KT_sb[DEP:2 * DEP, k0:k0 + P],
                        QT_sb[DEP:2 * DEP, g0:g0 + QB],
                        start=True, stop=True, tile_position=(DEP, 0))
                    ex = work.tile([P, 2 * QB], BF16, tag="exp", bufs=10,
                                   name=f"ex_{b}_{blk}_{kc}")
                    if kc in DVE_KCS:
                        with nc.allow_low_precision(
                                reason="fast-exp ripple, 2e-2 gate"):
                            nc.vector.tensor_scalar(
                                ex.bitcast(mybir.dt.int16), sc, FE_A, FE_B,
                                op0=mybir.AluOpType.mult,
                                op1=mybir.AluOpType.add)
                    else:
                        nc.scalar.activation(ex, sc, AF.Exp, scale=scale)
                    exp_tiles[(b, blk, kc)] = ex
                    if pv_from is not None:
                        # PV lags one kc so the next chunk's scores + exp
                        # issue before the PE stalls on this exp: exps on
                        # ACT and DVE then overlap across chunks
                        if kc > pv_from:
                            emit_pv(b, blk, kc - 1, kc)
                    elif pv:
                        emit_pv(b, blk, kc, kc + 1)

            def emit_pv(b, blk, kc_lo, kc_hi):
                t = attn_tiles[(b, blk)]
                for kc in range(kc_lo, kc_hi):
                    ex = exp_tiles.pop((b, blk, kc))
                    for c in range(2):
                        for h in range(HPC):
                            nc.tensor.matmul(
                                t[:, c, h * 65:(h + 1) * 65],
                                ex[:, h * QB + c * P:h * QB + (c + 1) * P],
                                V_sb[:, (b * NKC + kc) * HPC + h, :],
                                start=(kc == 0 and c == 0 and h == 0),
                                stop=(kc == NKC - 1),
                                skip_group_check=True)

            def emit_normalize(b, blk):
                # per-partition (per-query) scalar normalize straight into
                # the bf16 staging tile of the 512-row reshard pair
                t = attn_tiles.pop((b, blk))
                pair = blk // 2
                if (b, pair) not in an_tiles:
                    an_tiles[(b, pair)] = work.tile(
                        [P, NQS, FPC], BF16, tag="attn_n", bufs=3,
                        name=f"an_{b}_{pair}")
                an = an_tiles[(b, pair)]
                rc = work.tile([P, 2, 2], F32, tag="rc", bufs=2,
                               name=f"rc_{b}_{blk}")
                with nc.allow_low_precision(reason="recip of f32 denom"):
                    nc.vector.reciprocal(rc[:, :, 0], t[:, :, 0])
                    nc.vector.reciprocal(rc[:, :, 1], t[:, :, 65])
                with nc.allow_low_precision(
                        reason="bf16 reshard payload, 2e-2 gate"):
                    for c in range(2):
                        for h in range(HPC):
                            nc.vector.tensor_scalar_mul(
                                an[:, (blk % 2) * 2 + c,
                                   h * DEP:(h + 1) * DEP],
                                t[:, c, h * 65 + 1:(h + 1) * 65],
                                rc[:, c, h:h + 1])

            def emit_staging(b, pair):
                gi, k = grp_of[b * NQC + pair]
                an = an_tiles.pop((b, pair))
                nc.sync.dma_start(a2a_in[gi][:, :, k], an)

            def emit_collective(gi):
                nc.gpsimd.collective_compute(
                    "AllToAll", mybir.AluOpType.bypass,
                    replica_groups=[list(range(ncores))],
                    ins=[a2a_in[gi].rearrange(
                        "phi jj ch ll s f -> (phi jj) (ch ll s f)")],
                    outs=[a2a_out[gi].rearrange(
                        "src ch r f -> src (ch r f)")])

            cq_tiles = {}

            def emit_cq_group(gi):
                # ONE transposing XBAR load per collective group:
                # [src*ch*64 rows, 128 f] -> [128 f, (src ch), 64].
                # SP-issued (XBAR needs a HWDGE queue; ACT would stall the
                # exp stream). SP is in-order, so these are placed where
                # their collective-wait predates the next SP item's dep.
                g = groups[gi]
                cqg = work.tile([FPC, ncores * len(g), GR], BF16,
                                tag=f"cqg{gi}", bufs=1, name=f"cqg_{gi}")
                nc.sync.dma_start(
                    cqg, a2a_out[gi].rearrange("src ch r f -> (src ch r) f"),
                    transpose=True)
                for k, ch in enumerate(g):
                    cq_tiles[ch] = (cqg, k, len(g))

            def emit_granule(ch, pool, tag, bufs, use_act=False,
                             split_store=False):
                cqg, k, glen = cq_tiles[ch]
                otg = work.tile([P, NDO, GR], F32, tag="otg", bufs=8,
                                name=f"otg_{ch}")
                dst = outT.ap()[:, ch * GR:(ch + 1) * GR].rearrange(
                    "(dd p) n -> p dd n", p=P)
                for do in range(NDO):
                    # full-bank accumulator (start=True zeroes whole bank)
                    pg = pool.tile([P, 512], F32, tag=tag, bufs=bufs,
                                   name=f"opg_{ch}_{do}")
                    for i in range(NDO):
                        nc.tensor.matmul(
                            pg[:, 0:GR],
                            wo_sb[:, i, do * P:(do + 1) * P],
                            cqg[:, i * glen + k, :],
                            start=(i == 0), stop=(i == NDO - 1))
                    if use_act and do % 2 == 1:
                        nc.scalar.activation(
                            otg[:, do, :], pg[:, 0:GR], AF.Identity,
                            bias=bo_sb[:, do:do + 1])
                    else:
                        nc.vector.tensor_scalar_add(
                            otg[:, do, :], pg[:, 0:GR], bo_sb[:, do:do + 1])
                    if split_store and do % 2 == 1:
                        nc.sync.dma_start(dst[:, do - 1:do + 1],
                                          otg[:, do - 1:do + 1, :])
                if not split_store:
                    nc.sync.dma_start(dst, otg)

            def emit_warmers(n, pool):
                warm = pool.tile([P, 512], F32, tag="warm", bufs=1,
                                 name="warm")
                for _ in range(n):
                    nc.tensor.matmul(warm, wo_sb[:, 0, 0:P],
                                     wo_sb[:, 0, 0:512],
                                     start=True, stop=True)

            # ================= schedule =================
            # batch 0 lead-in: blocks 0 AND 1 paced by the arriving K/V
            # groups (both only need Q(rwb=0)); the proj-dense window then
            # absorbs two blocks of exp work instead of idling ACT
            for rwb in range(NRWB):
                emit_proj(0, rwb, js=((0, 1) if rwb == 0 else (1,)))
                emit_attn(0, 0, rwb * KCG, (rwb + 1) * KCG, pv=False)
                emit_attn(0, 1, rwb * KCG, (rwb + 1) * KCG, pv=False)
                for tb in range(rwb * TBG, (rwb + 1) * TBG):
                    emit_trans(0, tb)
                emit_pv(0, 0, rwb * KCG, (rwb + 1) * KCG)
                emit_pv(0, 1, rwb * KCG, (rwb + 1) * KCG)
                if rwb == NRWB - 1:
                    emit_proj(0, 1, js=(0,))
            emit_normalize(0, 0)

            def emit_proj_group(b, rwb):
                emit_proj(b, rwb, js=((0, 1) if rwb == 0 else (1,)))
                for tb in range(rwb * TBG, (rwb + 1) * TBG):
                    emit_trans(b, tb)

            LA = 2
            coll_q = []

            def maybe_collective(ch):
                gi, k = grp_of[ch]
                if k == len(groups[gi]) - 1:
                    emit_collective(gi)
                    coll_q.append(gi)

            def pop_cq():
                # emit cq transposes for collectives emitted in earlier
                # iterations: their SP waits are (nearly) satisfied, and
                # emitting them before the NEXT collective keeps the
                # scheduler from ordering them after the whole tail
                while coll_q:
                    emit_cq_group(coll_q.pop(0))

            def steady_block(b, blk, extra=None):
                emit_attn(b, blk, 0, LA, pv=False)
                emit_normalize(b, blk - 1)
                if blk % 2 == 0:
                    pair = blk // 2 - 1
                    emit_staging(b, pair)
                    pop_cq()
                    maybe_collective(b * NQC + pair)
                emit_pv(b, blk, 0, LA)
                emit_attn(b, blk, LA, NKC // 2, pv_from=LA)
                if extra is not None:
                    extra()
                emit_attn(b, blk, NKC // 2, NKC, pv_from=NKC // 2 - 1)
                emit_pv(b, blk, NKC - 1, NKC)

            for blk in range(2, NQB):
                extra = None
                if blk % 2 == 1 and 1 < blk // 2 + 1 < NRWB:
                    extra = (lambda r: lambda: emit_proj(0, r, js=(0,)))(
                        blk // 2 + 1)
                elif blk == NQB - 1 and B > 1:
                    # batch 1's first K/V group, ahead of its block 0
                    extra = lambda: emit_proj_group(1, 0)
                steady_block(0, blk, extra)
            # batch boundary: batch 1's first scores ahead of batch 0's
            # final normalize+staging
            emit_attn(1, 0, 0, KCG, pv=False)
            emit_normalize(0, NQB - 1)
            emit_staging(0, NQC - 1)
            pop_cq()
            maybe_collective(NQC - 1)
            # output-projection weights stream in after all x chunks
            nc.sync.dma_start(
                wo_sb, woT.ap().rearrange("(dd p) m -> p dd m", p=P))
            for b in range(1, B):
                # K/V groups 1..3 arrive just-in-time across blocks 0 AND 1
                emit_attn(b, 1, 0, KCG, pv=False)
                emit_pv(b, 0, 0, KCG)
                emit_pv(b, 1, 0, KCG)
                for g in range(1, NRWB):
                    emit_proj_group(b, g)
                    emit_attn(b, 0, g * KCG, (g + 1) * KCG, pv=False)
                    emit_attn(b, 1, g * KCG, (g + 1) * KCG, pv=False)
                    emit_pv(b, 0, g * KCG, (g + 1) * KCG)
                    emit_pv(b, 1, g * KCG, (g + 1) * KCG)
                    if g == NRWB - 1:
                        emit_proj(b, 1, js=(0,))
                emit_normalize(b, 0)
                for blk in range(2, NQB):
                    extra = None
                    if blk % 2 == 1 and 1 < blk // 2 + 1 < NRWB:
                        extra = (lambda r: lambda: emit_proj(b, r,
                                                             js=(0,)))(
                            blk // 2 + 1)
                    steady_block(b, blk, extra)
                emit_normalize(b, NQB - 1)
                emit_staging(b, NQC - 1)
                pop_cq()
                maybe_collective(b * NQC + NQC - 1)
            psd.__exit__(None, None, None)

            # ---- output projection: granules stream as collectives land;
            # only the last chunk trails the last collective ----
            psf = tc.tile_pool(name="ps_f", bufs=1, space="PSUM")
            ps2 = psf.__enter__()
            done_cq = set()
            for ch in range(NCH):
                gi, k = grp_of[ch]
                if ch not in cq_tiles and gi not in done_cq:
                    emit_cq_group(gi)
                    done_cq.add(gi)
                last = ch == NCH - 1
                if last:
                    # hold the p-state through the last collective wait
                    emit_warmers(16, ps2)
                emit_granule(ch, ps2, "oproj", 6, use_act=True,
                             split_store=last)
            psf.__exit__(None, None, None)

    nc.finalize()
    return nc


# ---------------- host side ----------------

_NC_CACHE = {}

B, S, D, H = 2, 2048, 1024, 16
NCORES = 8


def _prep_inputs(x, Wq, bq, Wk, bk, Wv, bv, Wo, bo, ncores):
    Dl = x.shape[-1]
    R = x.shape[0] * x.shape[1]
    FPC = Dl // ncores
    NDO = Dl // P
    xT = np.ascontiguousarray(x.reshape(R, Dl).T).astype(ml_dtypes.bfloat16)
    woT = np.ascontiguousarray(Wo.T).astype(ml_dtypes.bfloat16)
    bo_f = bo + Wo.astype(np.float64) @ bv.astype(np.float64)
    bo2 = np.ascontiguousarray(
        bo_f.astype(np.float32).reshape(NDO, P).T)
    maps = []
    for c in range(ncores):
        fsl = slice(c * FPC, (c + 1) * FPC)
        wqkvT = np.ascontiguousarray(
            np.concatenate([Wq[fsl], Wk[fsl], Wv[fsl]], axis=0).T).astype(
            ml_dtypes.bfloat16)
        bqkv = np.ascontiguousarray(
            np.stack([bq[fsl], bk[fsl], bv[fsl]], axis=1))
        maps.append(dict(xT=xT, wqkvT=wqkvT, bqkv=bqkv, woT=woT, bo=bo2))
    return maps


def kernel(x, Wq, bq, Wk, bk, Wv, bv, Wo, bo):
    from concourse.bass_utils import run_bass_kernel_spmd

    args = [np.asarray(a, np.float32)
            for a in (x, Wq, bq, Wk, bk, Wv, bv, Wo, bo)]
    x = args[0]
    Bx, Sx, Dx = x.shape
    key = (Bx, Sx, Dx)
    if key not in _NC_CACHE:
        _NC_CACHE[key] = build_nc(B=Bx, S=Sx, D=Dx, H=H, ncores=NCORES)
    nc = _NC_CACHE[key]

    in_maps = _prep_inputs(*args, NCORES)
    trace = os.environ.get("KERNEL_TRACE", "0") == "1"
    try:
        res = run_bass_kernel_spmd(nc, in_maps, core_ids=list(range(NCORES)),
                                   trace=trace)
    except ModuleNotFoundError:
        res = run_bass_kernel_spmd(nc, in_maps, core_ids=list(range(NCORES)),
                                   trace=False)
    kernel._last_results = res
    QCH = min(512, Sx)
    NQC = Sx // QCH
    GR = QCH // NCORES
    NQS = QCH // P
    out = np.empty((Bx * Sx, Dx), np.float32)
    # receiver c, granule (b,qc), row r=4*ll+s  <->
    # q = 128*s + 64*phi + 16*jj + ll with c = 4*phi + jj
    ll, s = np.meshgrid(np.arange(16), np.arange(NQS), indexing="ij")
    rr = (4 * ll + s).ravel()                       # granule row index
    for c in range(NCORES):
        oc = res.results[c]["outT"].T              # [B*Sc, D]
        phi, jj = divmod(c, 4)
        qloc = (128 * s + 64 * phi + 16 * jj + ll).ravel()
        for b2 in range(Bx):
            for qc in range(NQC):
                g = b2 * NQC + qc
                out[b2 * Sx + qc * QCH + qloc] = oc[g * GR + rr]
    return np.ascontiguousarray(out).reshape(Bx, Sx, Dx)
